# revision 1
# baseline (speedup 1.0000x reference)
"""EntityDisambiguationHead Trainium2 kernel.

Computes out[b,s,e] = cos_sim(tanh(x @ W.T + b), entity_embedding[e]) for
B=4, S=128, D_in=768, D_e=256, E=100000, sharding the entity axis across
8 NeuronCores (each core handles 12544 = 98*128 entities, padded from 12500).

Per-core math (all on device):
  q  = tanh(x @ W.T + b)                [512, 256]
  a  = 1/sqrt(||q_t||^2 + eps)          per token
  qn = q * a                            row-normalized
  c  = 1/sqrt(||ent_e||^2 + eps)        per entity
  enT = (ent_tile)^T @ diag(c)          transposed + normalized via TensorE
  out = qn @ enT                        [512, 12544] (f32r matmuls)

Host side only shards/pads inputs and concatenates outputs.
"""

import os
from contextlib import ExitStack

import numpy as np

import concourse.bass as bass
import concourse.bacc as bacc
import concourse.mybir as mybir
import concourse.tile as tile
from concourse.masks import make_identity

F32 = mybir.dt.float32
F32R = mybir.dt.float32r
AF = mybir.ActivationFunctionType
ALU = mybir.AluOpType

N_CORES = 8
E_FULL = 100000
E_PER_CORE = E_FULL // N_CORES          # 12500
E_TILES = (E_PER_CORE + 127) // 128     # 98
E_PAD = E_TILES * 128                   # 12544
T = 512                                 # tokens = 4*128
D_IN = 768
D_E = 256
EPS2 = 1e-16                            # added under sqrt ~= max(norm, 1e-8)


def build_nc(e_tiles=E_TILES, mm_dt=F32R, tr_dt=F32):
    """Build the per-core Bass program (SPMD: same program on all cores)."""
    nc = bacc.Bacc("TRN2", target_bir_lowering=False, debug=False)

    x_d = nc.dram_tensor("x", [T, D_IN], F32, kind="ExternalInput").ap()
    w_d = nc.dram_tensor("w", [D_E, D_IN], F32, kind="ExternalInput").ap()
    b_d = nc.dram_tensor("b", [1, D_E], F32, kind="ExternalInput").ap()
    e_d = nc.dram_tensor("ent", [e_tiles * 128, D_E], F32, kind="ExternalInput").ap()
    o_d = nc.dram_tensor("out", [T, e_tiles * 128], F32, kind="ExternalOutput").ap()

    # DRAM views with the 128-partition dim innermost-of-rows
    x_v = x_d.rearrange("(tt p) d -> p tt d", p=128)      # [128, 4, 768]
    w_v = w_d.rearrange("(h p) d -> p h d", p=128)        # [128, 2, 768]
    e_v = e_d.rearrange("(j p) d -> p j d", p=128)        # [128, e_tiles, 256]
    o_v = o_d.rearrange("(tt p) e -> p tt e", p=128)      # [128, 4, E_PAD]

    # entity slice groups: 4 tiles (512 cols) each, possible smaller tail
    groups = []
    t0 = 0
    while t0 < e_tiles:
        n = min(4, e_tiles - t0)
        groups.append((t0, n))
        t0 += n

    with tile.TileContext(nc) as tc, ExitStack() as ctx:
        const = ctx.enter_context(tc.tile_pool(name="const", bufs=1))
        psumA = ctx.enter_context(tc.tile_pool(name="psumA", bufs=2, space="PSUM"))
        psumB = ctx.enter_context(tc.tile_pool(name="psumB", bufs=2, space="PSUM"))

        # ---------------- constants ----------------
        identity = const.tile([128, 128], F32)
        make_identity(nc, identity)
        identity_r = const.tile([128, 128], mm_dt)
        nc.vector.tensor_copy(identity_r, identity)
        ones_f = const.tile([1, 128], F32)
        nc.vector.memset(ones_f, 1.0)
        ones_row = const.tile([1, 128], mm_dt)
        nc.vector.tensor_copy(ones_row, ones_f)
        eps_col = const.tile([128, 1], F32)
        nc.vector.memset(eps_col, EPS2)
        b_f32 = const.tile([1, D_E], F32)
        nc.sync.dma_start(out=b_f32, in_=b_d)
        b_sb = const.tile([1, D_E], mm_dt)
        nc.vector.tensor_copy(b_sb, b_f32)
        qnt = const.tile([128, 2, T], mm_dt)  # q normalized, transposed: [d_half, h, t]

        ent_pool = ctx.enter_context(tc.tile_pool(name="ent_pool", bufs=10))
        entn_pool = ctx.enter_context(tc.tile_pool(name="entn_pool", bufs=2))
        sq_pool = ctx.enter_context(tc.tile_pool(name="sq_pool", bufs=2))
        small = ctx.enter_context(tc.tile_pool(name="small", bufs=6))
        enT_pool = ctx.enter_context(tc.tile_pool(name="enT_pool", bufs=8))
        out_pool = ctx.enter_context(tc.tile_pool(name="out_pool", bufs=3))
        # ---------------- main loop over entity slices ----------------
        # Two-stage software pipeline (emission skew SKEW slices):
        #   stage1: load -> norms -> scale -> transpose -> enT copy
        #   stage2: main GEMM -> psum pair copies (-> paired store)
        SKEW = 3

        def stage1(g0, ng):
            ent = ent_pool.tile([128, 4, D_E], F32, tag="ent", name="ent")
            nc.sync.dma_start(out=ent[:, 0:ng, :], in_=e_v[:, g0:g0 + ng, :])

            sq = sq_pool.tile([128, 4, D_E], F32, tag="sq", name="sq")
            nrm = small.tile([128, 4], F32, tag="nrm", name="nrm")
            nc.scalar.activation(
                sq.rearrange("p j d -> p (j d)")[:, 0:ng * D_E],
                ent.rearrange("p j d -> p (j d)")[:, 0:ng * D_E],
                AF.Square,
            )
            nc.vector.reduce_sum(
                nrm[:, 0:ng], sq[:, 0:ng, :], mybir.AxisListType.X
            )
            c = small.tile([128, 4], F32, tag="c", name="c")
            nc.scalar.activation(c[:, 0:ng], nrm[:, 0:ng], AF.Sqrt, bias=eps_col)
            nc.vector.reciprocal(c[:, 0:ng], c[:, 0:ng])

            pT = psumA.tile([128, 4, 2, 128], mm_dt, tag="pT", name="pT")
            ent_n = entn_pool.tile([128, 4, D_E], mm_dt, tag="ent_n", name="ent_n")
            for j in range(ng):
                nc.vector.tensor_scalar_mul(ent_n[:, j, :], ent[:, j, :], c[:, j:j + 1])
                for h in range(2):
                    nc.tensor.transpose(
                        pT[:, j, h, :],
                        ent_n[:, j, 128 * h:128 * (h + 1)],
                        identity_r,
                    )
            enT = enT_pool.tile([128, 2, 512], mm_dt, tag="enT", name="enT")
            nc.scalar.copy(
                enT.rearrange("p h (j e) -> p h j e", e=128)[:, :, 0:ng, :],
                pT.rearrange("p j h e -> p h j e")[:, :, 0:ng, :],
            )
            return enT

        def stage2(ng, enT, ob, ob_off):
            width = ng * 128
            for pr in range(2):
                po = psumB.tile([128, 2, 512], F32, tag="po", name="po")
                for i in range(2):
                    tt = 2 * pr + i
                    for h in range(2):
                        nc.tensor.matmul(
                            po[:, i, 0:width],
                            qnt[:, h, 128 * tt:128 * (tt + 1)],
                            enT[:, h, 0:width],
                            start=(h == 0),
                            stop=(h == 1),
                        )
                if pr == 0:
                    nc.vector.tensor_copy(
                        ob[:, 0:2, ob_off:ob_off + width], po[:, :, 0:width])
                else:
                    nc.scalar.copy(
                        ob[:, 2:4, ob_off:ob_off + width], po[:, :, 0:width])

        # pair slices for the output store; schedule stage1 SKEW pairs ahead
        pairs = []
        gi = 0
        while gi < len(groups):
            pairs.append(groups[gi:gi + 2])
            gi += 2
        # process the short tail pair early so the pipeline drains on a warm chain
        if len(pairs) > 1:
            pairs = [pairs[-1]] + pairs[:-1]

        # prologue: first PRO pairs' stage1 ahead of q-setup (keeps DMA streaming)
        PRO = min(4, len(pairs))
        enTs = {}
        for pi in range(PRO):
            enTs[pi] = [stage1(gg, ng) for (gg, ng) in pairs[pi]]

        # ---------------- setup: load x, W and transpose ----------------
        setup_ctx = ExitStack()
        setup = setup_ctx.enter_context(tc.tile_pool(name="setup", bufs=1))
        x_nat = setup.tile([128, 4, D_IN], F32)
        w_nat = setup.tile([128, 2, D_IN], F32)
        for h in range(2):
            nc.sync.dma_start(out=w_nat[:, h, :], in_=w_v[:, h, :])
        for tt in range(4):
            nc.sync.dma_start(out=x_nat[:, tt, :], in_=x_v[:, tt, :])

        xt = setup.tile([128, 6, T], mm_dt)    # [d_in_chunk, k, t]
        wt = setup.tile([128, 6, D_E], mm_dt)  # [d_in_chunk, k, d_e]
        for k in range(6):
            ps_w = psumA.tile([128, 1024], F32, tag="pT")
            for h in range(2):
                nc.tensor.transpose(
                    ps_w[:, h * 128:(h + 1) * 128],
                    w_nat[:, h, 128 * k:128 * (k + 1)],
                    identity,
                )
            nc.vector.tensor_copy(wt[:, k, :], ps_w[:, 0:D_E])
        for k in range(6):
            ps_x = psumA.tile([128, 1024], F32, tag="pT")
            for tt in range(4):
                nc.tensor.transpose(
                    ps_x[:, tt * 128:(tt + 1) * 128],
                    x_nat[:, tt, 128 * k:128 * (k + 1)],
                    identity,
                )
            nc.vector.tensor_copy(xt[:, k, :], ps_x[:, 0:T])

        # ---------------- q = tanh(x W^T + b), qn = q/||q|| ----------------
        q_sb = setup.tile([128, 4, D_E], F32)
        qn_sb = setup.tile([128, 4, D_E], F32)
        nrm_q = setup.tile([128, 4], F32)
        a_col = setup.tile([128, 4], F32)
        sqq = setup.tile([128, D_E], F32)
        for tt in range(4):
            psq = psumB.tile([128, 1024], F32, tag="po")
            for k in range(6):
                nc.tensor.matmul(
                    psq[:, 0:D_E],
                    xt[:, k, 128 * tt:128 * (tt + 1)],
                    wt[:, k, :],
                    start=(k == 0),
                    stop=False,
                )
            nc.tensor.matmul(
                psq[:, 0:D_E],
                ones_row,
                b_sb,
                start=False,
                stop=True,
            )
            nc.scalar.activation(q_sb[:, tt, :], psq[:, 0:D_E], AF.Tanh)
            nc.vector.tensor_mul(sqq, q_sb[:, tt, :], q_sb[:, tt, :])
            nc.vector.reduce_sum(
                nrm_q[:, tt:tt + 1], sqq, mybir.AxisListType.X
            )
        nc.scalar.activation(a_col, nrm_q, AF.Sqrt, bias=eps_col)
        nc.vector.reciprocal(a_col, a_col)
        for tt in range(4):
            nc.vector.tensor_scalar_mul(qn_sb[:, tt, :], q_sb[:, tt, :], a_col[:, tt:tt + 1])
        for h in range(2):
            ps_q = psumA.tile([128, 1024], F32, tag="pT")
            for tt in range(4):
                nc.tensor.transpose(
                    ps_q[:, tt * 128:(tt + 1) * 128],
                    qn_sb[:, tt, h * 128:(h + 1) * 128],
                    identity,
                )
            nc.vector.tensor_copy(qnt[:, h, :], ps_q[:, 0:T])

        setup_ctx.close()  # release setup SBUF

        for di in range(len(pairs)):
            pi = di + PRO
            if pi < len(pairs):
                enTs[pi] = [stage1(gg, ng) for (gg, ng) in pairs[pi]]
            if True:
                pair = pairs[di]
                pw = sum(ng for _, ng in pair) * 128
                g0 = pair[0][0]
                ob = out_pool.tile([128, 4, 1024], F32, tag="ob", name="ob")
                off = 0
                for (gg, ng), enT in zip(pair, enTs.pop(di)):
                    stage2(ng, enT, ob, off)
                    off += ng * 128
                nc.sync.dma_start(
                    out=o_v[:, 0:2, g0 * 128:g0 * 128 + pw], in_=ob[:, 0:2, 0:pw]
                )
                nc.sync.dma_start(
                    out=o_v[:, 2:4, g0 * 128:g0 * 128 + pw], in_=ob[:, 2:4, 0:pw]
                )

    nc.compile()
    return nc


_CACHE = {}


def _best_effort_device_reset():
    """Recover wedged NeuronCores (NRT_EXEC_UNIT_UNRECOVERABLE) if the axon
    PJRT library is present. Safe on a healthy device; done once per process
    before the first execution."""
    try:
        import ctypes

        if os.path.exists("/opt/axon/libaxon_pjrt.so"):
            lib = ctypes.CDLL("/opt/axon/libaxon_pjrt.so")
            if hasattr(lib, "axon_reset"):
                lib.axon_reset.restype = ctypes.c_int64
                lib.axon_reset()
    except Exception:
        pass


def _get_nc():
    if "nc" not in _CACHE:
        _best_effort_device_reset()
        _CACHE["nc"] = build_nc()
    return _CACHE["nc"]


def kernel(x, W, b, entity_embedding, trace=False):
    from concourse.bass_utils import run_bass_kernel_spmd

    nc = _get_nc()
    x2 = np.ascontiguousarray(np.asarray(x, dtype=np.float32).reshape(T, D_IN))
    w2 = np.ascontiguousarray(np.asarray(W, dtype=np.float32))
    b2 = np.ascontiguousarray(np.asarray(b, dtype=np.float32).reshape(1, D_E))
    ent = np.asarray(entity_embedding, dtype=np.float32)

    pad = np.ones((E_PAD - E_PER_CORE, D_E), dtype=np.float32)
    in_maps = []
    for i in range(N_CORES):
        shard = np.ascontiguousarray(
            np.concatenate([ent[i * E_PER_CORE:(i + 1) * E_PER_CORE], pad], axis=0)
        )
        in_maps.append({"x": x2, "w": w2, "b": b2, "ent": shard})

    res = run_bass_kernel_spmd(nc, in_maps, core_ids=list(range(N_CORES)), trace=trace)
    kernel.last = res
    outs = [res.results[i]["out"][:, :E_PER_CORE] for i in range(N_CORES)]
    full = np.concatenate(outs, axis=1).reshape(4, 128, E_FULL)
    return np.ascontiguousarray(full.astype(np.float32))


kernel.last = None



# revision 8
# speedup vs baseline: 1.3380x; 1.3380x over previous
"""EntityDisambiguationHead Trainium2 kernel (bf16 rewrite).

Computes out[b,s,e] = cos_sim(tanh(x @ W.T + b), entity_embedding[e]) for
B=4, S=128, D_in=768, D_e=256, E=100000, sharding the entity axis across
8 NeuronCores (each core handles 12544 = 98*128 entities, padded from 12500).

All wire traffic is bf16 (host casts + relayouts); GEMMs run bf16 on the PE
with f32 PSUM accumulation. Entity L2-normalization is fused into the PE
transpose by multiplying with diag(1/||e||) instead of the identity:

  per core:
    q   = tanh(x @ W.T + b)               [512, 256]   (PE, bf16)
    a   = 1/sqrt(||q_t||^2 + eps)         per token
    qnt = (q^T scaled by a)               via PE matmul vs diag(a)
    c   = 1/sqrt(||ent_e||^2 + eps)       per entity (sq+reduce+rsqrt)
    enT = ent_tile^T @ diag(c)            transposed + normalized on PE
    out = qnt^T @ enT -> bf16             [512, 12544]

Host side only shards/casts/relayouts inputs and concatenates outputs.
DMA: entity loads stream on the scalar HWDGE ring (7 big contiguous
transfers issued upfront), stores go out on the sync ring.
"""

import os
from contextlib import ExitStack

import numpy as np
from ml_dtypes import bfloat16

import concourse.bass as bass
import concourse.bacc as bacc
import concourse.mybir as mybir
import concourse.tile as tile
from concourse.masks import make_identity

F32 = mybir.dt.float32
BF16 = mybir.dt.bfloat16
AF = mybir.ActivationFunctionType
ALU = mybir.AluOpType

N_CORES = 8
E_FULL = 100000
E_PER_CORE = E_FULL // N_CORES          # 12500
E_TILES = (E_PER_CORE + 127) // 128     # 98
E_PAD = E_TILES * 128                   # 12544
T = 512                                 # tokens = 4*128
D_IN = 768
D_E = 256
EPS2 = 1e-16                            # added under sqrt ~= max(norm, 1e-8)

CHUNK = 16                              # entity tiles per load DMA


def build_nc():
    """Build the per-core Bass program (SPMD: same program on all cores)."""
    nc = bacc.Bacc("TRN2", target_bir_lowering=False, debug=False)

    # DRAM tensors in device-friendly (partition-major) layouts; host prepares.
    x_d = nc.dram_tensor("x", [128, 6 * T], BF16, kind="ExternalInput").ap()
    w_d = nc.dram_tensor("w", [128, 6 * D_E], BF16, kind="ExternalInput").ap()
    b_d = nc.dram_tensor("b", [1, D_E], BF16, kind="ExternalInput").ap()
    e_d = nc.dram_tensor("ent", [128, E_TILES * D_E], BF16, kind="ExternalInput").ap()
    o_d = nc.dram_tensor("out", [128, 4 * E_PAD], BF16, kind="ExternalOutput").ap()

    x_v = x_d.rearrange("p (k t) -> p k t", t=T)          # [128, 6, 512] = x^T
    w_v = w_d.rearrange("p (k e) -> p k e", e=D_E)        # [128, 6, 256] = W^T
    e_v = e_d.rearrange("p (j d) -> p j d", d=D_E)        # [128, 98, 256]
    o_v = o_d.rearrange("p (tt e) -> p tt e", e=E_PAD)    # [128, 4, 12544]

    # entity groups of 4 tiles (512 cols), tail group of 2
    groups = []
    t0 = 0
    while t0 < E_TILES:
        n = min(4, E_TILES - t0)
        groups.append((t0, n))
        t0 += n
    # pairs of groups -> 1024-wide output stores (last pair may be short)
    pairs = []
    gi = 0
    while gi < len(groups):
        pairs.append(groups[gi:gi + 2])
        gi += 2

    with tile.TileContext(nc) as tc, ExitStack() as ctx:
        const = ctx.enter_context(tc.tile_pool(name="const", bufs=1))
        psumA = ctx.enter_context(tc.tile_pool(name="psumA", bufs=2, space="PSUM"))
        psumB = ctx.enter_context(tc.tile_pool(name="psumB", bufs=2, space="PSUM"))

        # ---------------- constants ----------------
        identity_f = const.tile([128, 128], F32)
        make_identity(nc, identity_f)
        ident = const.tile([128, 128], BF16)
        nc.vector.tensor_copy(ident, identity_f)
        ones_f = const.tile([1, 128], F32)
        nc.vector.memset(ones_f, 1.0)
        ones_row = const.tile([1, 128], BF16)
        nc.vector.tensor_copy(ones_row, ones_f)
        eps_col = const.tile([128, 1], F32)
        nc.vector.memset(eps_col, EPS2)
        b_sb = const.tile([1, D_E], BF16)
        nc.sync.dma_start(out=b_sb, in_=b_d)
        qnt = const.tile([128, 2, T], BF16)   # q normalized+transposed [d, h, t]

        ent_pool = ctx.enter_context(tc.tile_pool(name="ent_pool", bufs=7))
        enT_pool = ctx.enter_context(tc.tile_pool(name="enT_pool", bufs=8))
        sq_pool = ctx.enter_context(tc.tile_pool(name="sq_pool", bufs=3))
        small = ctx.enter_context(tc.tile_pool(name="small", bufs=8))
        d_pool = ctx.enter_context(tc.tile_pool(name="d_pool", bufs=3))
        out_pool = ctx.enter_context(tc.tile_pool(name="out_pool", bufs=3))

        # ---------------- entity loads: big contiguous DMAs, issued upfront --
        chunk_tiles = []        # (j_start, n_tiles, tile)
        j0 = 0
        while j0 < E_TILES:
            n = min(CHUNK, E_TILES - j0)
            ct = ent_pool.tile([128, n, D_E], BF16, tag="ent", name=f"ent{j0}")
            nc.scalar.dma_start(out=ct, in_=e_v[:, j0:j0 + n, :])
            chunk_tiles.append((j0, n, ct))
            j0 += n

        def ent_slice(j_start, n):
            ci = j_start // CHUNK
            c0, cn, ct = chunk_tiles[ci]
            lo = j_start - c0
            assert lo + n <= cn
            return ct[:, lo:lo + n, :]

        # ---------------- per-pair stage 1: norms + scaled transpose --------
        def stage1(pair, pi):
            njs = [ng for _, ng in pair]
            jtot = sum(njs)
            nrm = small.tile([128, 8], F32, tag="nrm", name="nrm")
            joff = 0
            for k, (g0, ng) in enumerate(pair):
                ent = ent_slice(g0, ng)
                sq = sq_pool.tile([128, 4, D_E], BF16, tag="sq", name="sq")
                sqv = sq[:, 0:ng, :]
                if (pi + k) % 2 == 0:
                    nc.scalar.activation(sqv, ent, AF.Square)
                else:
                    nc.vector.tensor_mul(sqv, ent, ent)
                nc.vector.reduce_sum(
                    nrm[:, joff:joff + ng], sqv, mybir.AxisListType.X)
                joff += ng
            s = small.tile([128, 8], F32, tag="s", name="s")
            nc.scalar.activation(s[:, 0:jtot], nrm[:, 0:jtot], AF.Sqrt,
                                 bias=eps_col)
            c_bf = small.tile([128, 8], F32, tag="c", name="c")
            nc.vector.reciprocal(c_bf[:, 0:jtot], s[:, 0:jtot])
            dmat = d_pool.tile([128, 8, 128], BF16, tag="D", name="D")
            for j in range(jtot):
                nc.vector.tensor_scalar_mul(
                    dmat[:, j, :], ident, c_bf[:, j:j + 1])
            enTs = []
            joff = 0
            for k, (g0, ng) in enumerate(pair):
                ent = ent_slice(g0, ng)
                pT = psumA.tile([128, 4, 2, 128], F32, tag="pT", name="pT")
                for jj in range(ng):
                    for h in range(2):
                        nc.tensor.matmul(
                            pT[:, jj, h, :],
                            ent[:, jj, 128 * h:128 * (h + 1)],
                            dmat[:, joff + jj, :],
                            start=True, stop=True,
                        )
                enT = enT_pool.tile([128, 2, 512], BF16, tag="enT", name="enT")
                cp = nc.scalar.copy if (pi + k) % 2 == 0 else nc.vector.tensor_copy
                cp(
                    enT.rearrange("p h (j e) -> p h j e", e=128)[:, :, 0:ng, :],
                    pT.rearrange("p j h e -> p h j e")[:, :, 0:ng, :],
                )
                enTs.append(enT)
                joff += ng
            return enTs

        # ---------------- per-pair stage 2: main GEMM + store ---------------
        def stage2(pair, enTs, pi):
            pw = sum(ng for _, ng in pair) * 128
            g0 = pair[0][0]
            ob = out_pool.tile([128, 4, 1024], BF16, tag="ob", name="ob")
            for pr in range(2):
                off = 0
                for (gg, ng), enT in zip(pair, enTs):
                    width = ng * 128
                    po = psumB.tile([128, 2, 512], F32, tag="po", name="po")
                    for i in range(2):
                        tt = 2 * pr + i
                        for h in range(2):
                            nc.tensor.matmul(
                                po[:, i, 0:width],
                                qnt[:, h, 128 * tt:128 * (tt + 1)],
                                enT[:, h, 0:width],
                                start=(h == 0),
                                stop=(h == 1),
                            )
                    cp = (nc.scalar.copy if (pi + pr) % 2 == 0
                          else nc.vector.tensor_copy)
                    cp(ob[:, 2 * pr:2 * pr + 2, off:off + width],
                       po[:, :, 0:width])
                    off += width
            nc.sync.dma_start(
                out=o_v[:, :, g0 * 128:g0 * 128 + pw], in_=ob[:, :, 0:pw])

        # ---------------- prologue: stage1 for first SKEW pairs -------------
        SKEW = 3
        PRO = min(SKEW, len(pairs))
        enTs_map = {}
        for pi in range(PRO):
            enTs_map[pi] = stage1(pairs[pi], pi)

        # ---------------- setup: load x/W, q = tanh(xW^T+b), qnt ------------
        setup_ctx = ExitStack()
        setup = setup_ctx.enter_context(tc.tile_pool(name="setup", bufs=1))
        xt = setup.tile([128, 6, T], BF16)     # [d_in_chunk, k, t]
        wt = setup.tile([128, 6, D_E], BF16)   # [d_in_chunk, k, d_e]
        nc.sync.dma_start(out=xt, in_=x_v)
        nc.sync.dma_start(out=wt, in_=w_v)

        q_sb = setup.tile([128, 4, D_E], BF16)
        sqq = setup.tile([128, D_E], BF16)
        nrm_q = setup.tile([128, 4], F32)
        s_q = setup.tile([128, 4], F32)
        a_bf = setup.tile([128, 4], F32)
        for tt in range(4):
            psq = psumB.tile([128, 2, 512], F32, tag="po")
            pq = psq.rearrange("p a b -> p (a b)")
            for k in range(6):
                nc.tensor.matmul(
                    pq[:, 0:D_E],
                    xt[:, k, 128 * tt:128 * (tt + 1)],
                    wt[:, k, :],
                    start=(k == 0),
                    stop=False,
                )
            nc.tensor.matmul(pq[:, 0:D_E], ones_row, b_sb,
                             start=False, stop=True)
            nc.scalar.activation(q_sb[:, tt, :], pq[:, 0:D_E], AF.Tanh)
            nc.vector.tensor_mul(sqq, q_sb[:, tt, :], q_sb[:, tt, :])
            nc.vector.reduce_sum(
                nrm_q[:, tt:tt + 1], sqq, mybir.AxisListType.X)
        nc.scalar.activation(s_q, nrm_q, AF.Sqrt, bias=eps_col)
        nc.vector.reciprocal(a_bf, s_q)
        d_a = setup.tile([128, 4, 128], BF16)
        for tt in range(4):
            nc.vector.tensor_scalar_mul(d_a[:, tt, :], ident, a_bf[:, tt:tt + 1])
        ps_q = psumA.tile([128, 4, 2, 128], F32, tag="pT")
        pq_v = ps_q.rearrange("p a b c -> p (a b) c")  # [128, 8, 128]
        for h in range(2):
            for tt in range(4):
                nc.tensor.matmul(
                    pq_v[:, 4 * h + tt, :],
                    q_sb[:, tt, 128 * h:128 * (h + 1)],
                    d_a[:, tt, :],
                    start=True, stop=True,
                )
        nc.vector.tensor_copy(
            qnt.rearrange("p h (tt t) -> p h tt t", t=128),
            pq_v.rearrange("p (h tt) t -> p h tt t", h=2),
        )
        setup_ctx.close()  # release setup SBUF

        # ---------------- steady state ----------------
        for di in range(len(pairs)):
            pi = di + PRO
            if pi < len(pairs):
                enTs_map[pi] = stage1(pairs[pi], pi)
            stage2(pairs[di], enTs_map.pop(di), di)

    nc.compile()
    return nc


_CACHE = {}


def _best_effort_device_reset():
    """Recover wedged NeuronCores if the axon PJRT library is present."""
    try:
        import ctypes

        if os.path.exists("/opt/axon/libaxon_pjrt.so"):
            lib = ctypes.CDLL("/opt/axon/libaxon_pjrt.so")
            if hasattr(lib, "axon_reset"):
                lib.axon_reset.restype = ctypes.c_int64
                lib.axon_reset()
    except Exception:
        pass


def _get_nc():
    if "nc" not in _CACHE:
        _best_effort_device_reset()
        _CACHE["nc"] = build_nc()
    return _CACHE["nc"]


def kernel(x, W, b, entity_embedding, trace=False):
    from concourse.bass_utils import run_bass_kernel_spmd

    nc = _get_nc()

    # x: [4,128,768] -> x^T p-major [128, 6, 512] (t = tt*128 + p)
    x2 = np.asarray(x, dtype=np.float32).reshape(T, D_IN)
    xt = np.ascontiguousarray(
        x2.T.reshape(6, 128, T).transpose(1, 0, 2)
    ).astype(bfloat16).reshape(128, 6 * T)
    # W: [256,768] -> W^T p-major [128, 6, 256]
    w2 = np.asarray(W, dtype=np.float32)
    wt = np.ascontiguousarray(
        w2.T.reshape(6, 128, D_E).transpose(1, 0, 2)
    ).astype(bfloat16).reshape(128, 6 * D_E)
    b2 = np.asarray(b, dtype=np.float32).reshape(1, D_E).astype(bfloat16)
    ent = np.asarray(entity_embedding, dtype=np.float32)

    pad = np.ones((E_PAD - E_PER_CORE, D_E), dtype=np.float32)
    in_maps = []
    for i in range(N_CORES):
        shard = np.concatenate(
            [ent[i * E_PER_CORE:(i + 1) * E_PER_CORE], pad], axis=0
        ).astype(bfloat16)
        shard = np.ascontiguousarray(
            shard.reshape(E_TILES, 128, D_E).transpose(1, 0, 2)
        ).reshape(128, E_TILES * D_E)
        in_maps.append({"x": xt, "w": wt, "b": b2, "ent": shard})

    res = run_bass_kernel_spmd(nc, in_maps, core_ids=list(range(N_CORES)),
                               trace=trace)
    kernel.last = res
    outs = []
    for i in range(N_CORES):
        o = np.asarray(res.results[i]["out"]).reshape(128, 4, E_PAD)
        o = o.transpose(1, 0, 2).reshape(T, E_PAD)[:, :E_PER_CORE]
        outs.append(o)
    full = np.concatenate(outs, axis=1).astype(np.float32)
    return np.ascontiguousarray(full.reshape(4, 128, E_FULL))


kernel.last = None


# revision 10
# speedup vs baseline: 1.3629x; 1.0186x over previous
"""EntityDisambiguationHead Trainium2 kernel (bf16 rewrite).

Computes out[b,s,e] = cos_sim(tanh(x @ W.T + b), entity_embedding[e]) for
B=4, S=128, D_in=768, D_e=256, E=100000, sharding the entity axis across
8 NeuronCores (each core handles 12544 = 98*128 entities, padded from 12500).

All wire traffic is bf16 (host casts + relayouts); GEMMs run bf16 on the PE
with f32 PSUM accumulation. Entity L2-normalization is fused into the PE
transpose by multiplying with diag(1/||e||) instead of the identity:

  per core:
    q   = tanh(x @ W.T + b)               [512, 256]   (PE, bf16)
    a   = 1/sqrt(||q_t||^2 + eps)         per token
    qnt = (q^T scaled by a)               via PE matmul vs diag(a)
    c   = 1/sqrt(||ent_e||^2 + eps)       per entity (sq+reduce+rsqrt)
    enT = ent_tile^T @ diag(c)            transposed + normalized on PE
    out = qnt^T @ enT -> bf16             [512, 12544]

Host side only shards/casts/relayouts inputs and concatenates outputs.
DMA: entity loads stream on the scalar HWDGE ring (7 big contiguous
transfers issued upfront), stores go out on the sync ring.
"""

import os
from contextlib import ExitStack

import numpy as np
from ml_dtypes import bfloat16

import concourse.bass as bass
import concourse.bacc as bacc
import concourse.mybir as mybir
import concourse.tile as tile
from concourse.masks import make_identity

F32 = mybir.dt.float32
BF16 = mybir.dt.bfloat16
AF = mybir.ActivationFunctionType
ALU = mybir.AluOpType

N_CORES = 8
E_FULL = 100000
E_PER_CORE = E_FULL // N_CORES          # 12500
E_TILES = (E_PER_CORE + 127) // 128     # 98
E_PAD = E_TILES * 128                   # 12544
T = 512                                 # tokens = 4*128
D_IN = 768
D_E = 256
EPS2 = 1e-16                            # added under sqrt ~= max(norm, 1e-8)

CHUNK = 16                              # entity tiles per load DMA


def build_nc():
    """Build the per-core Bass program (SPMD: same program on all cores)."""
    nc = bacc.Bacc("TRN2", target_bir_lowering=False, debug=False)

    # DRAM tensors in device-friendly (partition-major) layouts; host prepares.
    x_d = nc.dram_tensor("x", [128, 6 * T], BF16, kind="ExternalInput").ap()
    w_d = nc.dram_tensor("w", [128, 6 * D_E], BF16, kind="ExternalInput").ap()
    b_d = nc.dram_tensor("b", [1, D_E], BF16, kind="ExternalInput").ap()
    e_d = nc.dram_tensor("ent", [128, E_TILES * D_E], BF16, kind="ExternalInput").ap()
    o_d = nc.dram_tensor("out", [128, 4 * E_PAD], BF16, kind="ExternalOutput").ap()

    x_v = x_d.rearrange("p (k t) -> p k t", t=T)          # [128, 6, 512] = x^T
    w_v = w_d.rearrange("p (k e) -> p k e", e=D_E)        # [128, 6, 256] = W^T
    e_v = e_d.rearrange("p (j d) -> p j d", d=D_E)        # [128, 98, 256]
    o_v = o_d.rearrange("p (tt e) -> p tt e", e=E_PAD)    # [128, 4, 12544]

    # entity groups of 4 tiles (512 cols), tail group of 2
    groups = []
    t0 = 0
    while t0 < E_TILES:
        n = min(4, E_TILES - t0)
        groups.append((t0, n))
        t0 += n
    # pairs of groups -> 1024-wide output stores (last pair may be short)
    pairs = []
    gi = 0
    while gi < len(groups):
        pairs.append(groups[gi:gi + 2])
        gi += 2

    with tile.TileContext(nc) as tc, ExitStack() as ctx:
        const = ctx.enter_context(tc.tile_pool(name="const", bufs=1))
        psumA = ctx.enter_context(tc.tile_pool(name="psumA", bufs=2, space="PSUM"))
        psumB = ctx.enter_context(tc.tile_pool(name="psumB", bufs=2, space="PSUM"))

        # ---------------- constants ----------------
        identity_f = const.tile([128, 128], F32)
        make_identity(nc, identity_f)
        ident = const.tile([128, 128], BF16)
        nc.vector.tensor_copy(ident, identity_f)
        ones_f = const.tile([1, 128], F32)
        nc.vector.memset(ones_f, 1.0)
        ones_row = const.tile([1, 128], BF16)
        nc.vector.tensor_copy(ones_row, ones_f)
        eps_col = const.tile([128, 1], F32)
        nc.vector.memset(eps_col, EPS2)
        b_sb = const.tile([1, D_E], BF16)
        nc.sync.dma_start(out=b_sb, in_=b_d)
        qnt = const.tile([128, 2, T], BF16)   # q normalized+transposed [d, h, t]

        ent_pool = ctx.enter_context(tc.tile_pool(name="ent_pool", bufs=7))
        enT_pool = ctx.enter_context(tc.tile_pool(name="enT_pool", bufs=8))
        sq_pool = ctx.enter_context(tc.tile_pool(name="sq_pool", bufs=3))
        small = ctx.enter_context(tc.tile_pool(name="small", bufs=8))
        d_pool = ctx.enter_context(tc.tile_pool(name="d_pool", bufs=3))
        out_pool = ctx.enter_context(tc.tile_pool(name="out_pool", bufs=3))

        # ---------------- entity loads: big contiguous DMAs, issued upfront --
        chunk_tiles = []        # (j_start, n_tiles, tile)
        j0 = 0
        while j0 < E_TILES:
            n = min(CHUNK, E_TILES - j0)
            ct = ent_pool.tile([128, n, D_E], BF16, tag="ent", name=f"ent{j0}")
            nc.scalar.dma_start(out=ct, in_=e_v[:, j0:j0 + n, :])
            chunk_tiles.append((j0, n, ct))
            j0 += n

        def ent_slice(j_start, n):
            ci = j_start // CHUNK
            c0, cn, ct = chunk_tiles[ci]
            lo = j_start - c0
            assert lo + n <= cn
            return ct[:, lo:lo + n, :]

        # ---------------- per-pair stage 1: norms + scaled transpose --------
        def stage1(pair, pi):
            jtot = sum(ng for _, ng in pair)
            ent_p = ent_slice(pair[0][0], jtot)   # pair is j-contiguous
            sq = sq_pool.tile([128, 8, D_E], BF16, tag="sq", name="sq")
            nc.vector.tensor_mul(sq[:, 0:jtot, :], ent_p, ent_p)
            nrm = small.tile([128, 8], F32, tag="nrm", name="nrm")
            nc.vector.reduce_sum(
                nrm[:, 0:jtot], sq[:, 0:jtot, :], mybir.AxisListType.X)
            s = small.tile([128, 8], F32, tag="s", name="s")
            nc.scalar.activation(s[:, 0:jtot], nrm[:, 0:jtot], AF.Sqrt,
                                 bias=eps_col)
            c_f = small.tile([128, 8], F32, tag="c", name="c")
            nc.vector.reciprocal(c_f[:, 0:jtot], s[:, 0:jtot])
            dmat = d_pool.tile([128, 8, 128], BF16, tag="D", name="D")
            nc.vector.tensor_mul(
                dmat[:, 0:jtot, :],
                identity_f[:, None, :].broadcast_to([128, jtot, 128]),
                c_f[:, 0:jtot, None].broadcast_to([128, jtot, 128]),
            )
            enTs = []
            joff = 0
            for k, (g0, ng) in enumerate(pair):
                ent = ent_slice(g0, ng)
                pT = psumA.tile([128, 4, 2, 128], F32, tag="pT", name="pT")
                for jj in range(ng):
                    for h in range(2):
                        nc.tensor.matmul(
                            pT[:, jj, h, :],
                            ent[:, jj, 128 * h:128 * (h + 1)],
                            dmat[:, joff + jj, :],
                            start=True, stop=True,
                        )
                enT = enT_pool.tile([128, 2, 512], BF16, tag="enT", name="enT")
                nc.scalar.copy(
                    enT.rearrange("p h (j e) -> p h j e", e=128)[:, :, 0:ng, :],
                    pT.rearrange("p j h e -> p h j e")[:, :, 0:ng, :],
                )
                enTs.append(enT)
                joff += ng
            return enTs

        # ---------------- per-pair stage 2: main GEMM + store ---------------
        def stage2(pair, enTs, pi):
            pw = sum(ng for _, ng in pair) * 128
            g0 = pair[0][0]
            ob = out_pool.tile([128, 4, 1024], BF16, tag="ob", name="ob")
            for pr in range(2):
                off = 0
                for (gg, ng), enT in zip(pair, enTs):
                    width = ng * 128
                    po = psumB.tile([128, 2, 512], F32, tag="po", name="po")
                    for i in range(2):
                        tt = 2 * pr + i
                        for h in range(2):
                            nc.tensor.matmul(
                                po[:, i, 0:width],
                                qnt[:, h, 128 * tt:128 * (tt + 1)],
                                enT[:, h, 0:width],
                                start=(h == 0),
                                stop=(h == 1),
                            )
                    # out-cast split: vector takes 1 of 4 (even pairs) to
                    # balance scalar (which also does all enT casts)
                    cp = (nc.vector.tensor_copy
                          if (pr == 1 and gg == pair[-1][0] and pi % 2 == 0)
                          else nc.scalar.copy)
                    cp(ob[:, 2 * pr:2 * pr + 2, off:off + width],
                       po[:, :, 0:width])
                    off += width
            nc.sync.dma_start(
                out=o_v[:, :, g0 * 128:g0 * 128 + pw], in_=ob[:, :, 0:pw])

        # ---------------- prologue: stage1 for first SKEW pairs -------------
        SKEW = 3
        PRO = min(SKEW, len(pairs))
        enTs_map = {}
        for pi in range(PRO):
            enTs_map[pi] = stage1(pairs[pi], pi)

        # ---------------- setup: load x/W, q = tanh(xW^T+b), qnt ------------
        setup_ctx = ExitStack()
        setup = setup_ctx.enter_context(tc.tile_pool(name="setup", bufs=1))
        xt = setup.tile([128, 6, T], BF16)     # [d_in_chunk, k, t]
        wt = setup.tile([128, 6, D_E], BF16)   # [d_in_chunk, k, d_e]
        nc.sync.dma_start(out=xt, in_=x_v)
        nc.sync.dma_start(out=wt, in_=w_v)

        q_sb = setup.tile([128, 4, D_E], BF16)
        sqq = setup.tile([128, D_E], BF16)
        nrm_q = setup.tile([128, 4], F32)
        s_q = setup.tile([128, 4], F32)
        a_bf = setup.tile([128, 4], F32)
        for tt in range(4):
            psq = psumB.tile([128, 2, 512], F32, tag="po")
            pq = psq.rearrange("p a b -> p (a b)")
            for k in range(6):
                nc.tensor.matmul(
                    pq[:, 0:D_E],
                    xt[:, k, 128 * tt:128 * (tt + 1)],
                    wt[:, k, :],
                    start=(k == 0),
                    stop=False,
                )
            nc.tensor.matmul(pq[:, 0:D_E], ones_row, b_sb,
                             start=False, stop=True)
            nc.scalar.activation(q_sb[:, tt, :], pq[:, 0:D_E], AF.Tanh)
            nc.vector.tensor_mul(sqq, q_sb[:, tt, :], q_sb[:, tt, :])
            nc.vector.reduce_sum(
                nrm_q[:, tt:tt + 1], sqq, mybir.AxisListType.X)
        nc.scalar.activation(s_q, nrm_q, AF.Sqrt, bias=eps_col)
        nc.vector.reciprocal(a_bf, s_q)
        d_a = setup.tile([128, 4, 128], BF16)
        for tt in range(4):
            nc.vector.tensor_scalar_mul(d_a[:, tt, :], ident, a_bf[:, tt:tt + 1])
        ps_q = psumA.tile([128, 4, 2, 128], F32, tag="pT")
        pq_v = ps_q.rearrange("p a b c -> p (a b) c")  # [128, 8, 128]
        for h in range(2):
            for tt in range(4):
                nc.tensor.matmul(
                    pq_v[:, 4 * h + tt, :],
                    q_sb[:, tt, 128 * h:128 * (h + 1)],
                    d_a[:, tt, :],
                    start=True, stop=True,
                )
        nc.vector.tensor_copy(
            qnt.rearrange("p h (tt t) -> p h tt t", t=128),
            pq_v.rearrange("p (h tt) t -> p h tt t", h=2),
        )
        setup_ctx.close()  # release setup SBUF

        # ---------------- steady state ----------------
        for di in range(len(pairs)):
            pi = di + PRO
            if pi < len(pairs):
                enTs_map[pi] = stage1(pairs[pi], pi)
            stage2(pairs[di], enTs_map.pop(di), di)

    nc.compile()
    return nc


_CACHE = {}


def _best_effort_device_reset():
    """Recover wedged NeuronCores if the axon PJRT library is present."""
    try:
        import ctypes

        if os.path.exists("/opt/axon/libaxon_pjrt.so"):
            lib = ctypes.CDLL("/opt/axon/libaxon_pjrt.so")
            if hasattr(lib, "axon_reset"):
                lib.axon_reset.restype = ctypes.c_int64
                lib.axon_reset()
    except Exception:
        pass


def _get_nc():
    if "nc" not in _CACHE:
        _best_effort_device_reset()
        _CACHE["nc"] = build_nc()
    return _CACHE["nc"]


def kernel(x, W, b, entity_embedding, trace=False):
    from concourse.bass_utils import run_bass_kernel_spmd

    nc = _get_nc()

    # x: [4,128,768] -> x^T p-major [128, 6, 512] (t = tt*128 + p)
    x2 = np.asarray(x, dtype=np.float32).reshape(T, D_IN)
    xt = np.ascontiguousarray(
        x2.T.reshape(6, 128, T).transpose(1, 0, 2)
    ).astype(bfloat16).reshape(128, 6 * T)
    # W: [256,768] -> W^T p-major [128, 6, 256]
    w2 = np.asarray(W, dtype=np.float32)
    wt = np.ascontiguousarray(
        w2.T.reshape(6, 128, D_E).transpose(1, 0, 2)
    ).astype(bfloat16).reshape(128, 6 * D_E)
    b2 = np.asarray(b, dtype=np.float32).reshape(1, D_E).astype(bfloat16)
    ent = np.asarray(entity_embedding, dtype=np.float32)

    pad = np.ones((E_PAD - E_PER_CORE, D_E), dtype=np.float32)
    in_maps = []
    for i in range(N_CORES):
        shard = np.concatenate(
            [ent[i * E_PER_CORE:(i + 1) * E_PER_CORE], pad], axis=0
        ).astype(bfloat16)
        shard = np.ascontiguousarray(
            shard.reshape(E_TILES, 128, D_E).transpose(1, 0, 2)
        ).reshape(128, E_TILES * D_E)
        in_maps.append({"x": xt, "w": wt, "b": b2, "ent": shard})

    res = run_bass_kernel_spmd(nc, in_maps, core_ids=list(range(N_CORES)),
                               trace=trace)
    kernel.last = res
    outs = []
    for i in range(N_CORES):
        o = np.asarray(res.results[i]["out"]).reshape(128, 4, E_PAD)
        o = o.transpose(1, 0, 2).reshape(T, E_PAD)[:, :E_PER_CORE]
        outs.append(o)
    full = np.concatenate(outs, axis=1).astype(np.float32)
    return np.ascontiguousarray(full.reshape(4, 128, E_FULL))


kernel.last = None
